# revision 53
# baseline (speedup 1.0000x reference)
"""Trainium2 Bass kernel for the 2-branch GCN+GAT+GraphNorm network.

Strategy (8 NeuronCores, SPMD):
  - Nodes partitioned contiguously: core c owns global nodes [c*NLOC,(c+1)*NLOC).
  - Node features for gathers live in an AllGathered bf16 table `hfull`
    [8*12544, 128] (256B rows).
  - Edges partitioned by dst core, grouped by dst window w = dstl//128 (98
    windows) and src quarter q = srow//25088 (int16-indexable subtables),
    sorted by src within each (w,q) run (DMA locality), padded to a uniform
    T_cap tiles per run (pad idx = last valid idx, drel -1).
  - Slot layout is chunk-major (chunk = CW windows, quarter-major inside), so
    bulk `dma_gather` calls of <=1024 indices stay within one quarter
    subtable. Calls round-robin 4 SWDGE queues.
  - Edge phase is batched: per chunk, one DVE mul+reduce computes a_s for all
    edges, per-window ST one-hots come from a single broadcast DMA + one
    is_equal; per-window pade matmuls give a_d per edge; leaky/exp/alpha-mul
    are chunk-wide DVE/ACT ops; aggregation is PSUM-accumulated one-hot
    matmuls producing [128 dst, 128+H] (numerator | softmax denominator).
  - Self-loop contributions are added in the node phase from purely local
    data.
  - Node phases are batched over blocks of TB=7 windows; per-graph stats are
    accumulated in PSUM across all windows with one-hot matmuls; GraphNorm is
    applied as x*A[g]-Bv[g] with per-graph coefficient rows broadcast by
    one-hot matmuls (transposed one-hots built from a host-provided row-major
    batch table, no PE transposes); exp/gelu ACT ops are grouped to avoid
    activation-table reloads.
  - GraphNorm moments use one 64x256 AllReduce per norm.
  - Branches interleaved so node phases/collectives of one branch can overlap
    the other branch's edge phase.
"""
import os
import numpy as np
import ml_dtypes

_ABL = set(filter(None, os.environ.get("ABL", "").split(",")))

P = 128
NCORES = 8
G = 64               # graphs per batch
H = 4                # attention heads
EPS = 1e-5
Q = 4                # src quarters (int16 subtable limit)
CW = 4               # windows per chunk
GCALL = 1024         # max idx per dma_gather (SWDGE ring)
NQUEUE = 4
TB = 7               # node-phase window block

bfl = ml_dtypes.bfloat16


# ----------------------------------------------------------------------------
# Host-side graph preprocessing
# ----------------------------------------------------------------------------

def _chunk_sizes(nt):
    sizes = []
    w = 0
    while w < nt:
        sizes.append(min(CW, nt - w))
        w += CW
    return sizes


def _prep_branch(edge_index, n, nloc, nloc_pad):
    """Per-core (w, q)-run edge lists, sorted by src within runs.

    Returns per-core dict with sorted srel/drel/norm arrays + run counts,
    plus normself (dis^2 per node).
    """
    src = edge_index[0].astype(np.int64)
    dst = edge_index[1].astype(np.int64)
    deg = np.bincount(dst, minlength=n).astype(np.float64) + 1.0  # + self loop
    dis = 1.0 / np.sqrt(deg)
    norm = (dis[src] * dis[dst]).astype(np.float32)
    normself = (dis * dis).astype(np.float32)
    srow = (src // nloc) * nloc_pad + (src % nloc)
    sub = (NCORES * nloc_pad) // Q
    nt = nloc_pad // P
    cores = []
    max_run = 0
    for c in range(NCORES):
        m = (dst >= c * nloc) & (dst < (c + 1) * nloc)
        s, d, wgt = srow[m], dst[m] - c * nloc, norm[m]
        q = s // sub
        w = d // P
        order = np.lexsort((s, q, w))
        s, d, wgt, q, w = s[order], d[order], wgt[order], q[order], w[order]
        counts = np.bincount(w * Q + q, minlength=nt * Q)
        max_run = max(max_run, counts.max())
        cores.append(dict(srel=(s - q * sub).astype(np.int64),
                          drel=(d % P).astype(np.float32),
                          wgt=wgt, counts=counts))
    return cores, normself, int(max_run)


def _pack_branch(cores, nt, t_cap):
    """Pack runs into the chunk-major slot stream; build device tables."""
    csizes = _chunk_sizes(nt)
    ncols = Q * nt * t_cap
    nslot = ncols * P
    idx16 = np.zeros((NCORES, 128, nslot // 16), np.int16)
    dstf = np.empty((NCORES, P, ncols), bfl)
    ewt = np.empty((NCORES, P, ncols), bfl)
    dsttr = np.empty((NCORES, ncols, P), bfl)

    # col base for (w, q): chunk-major
    chunk_base = np.zeros(len(csizes) + 1, np.int64)
    for i, cs in enumerate(csizes):
        chunk_base[i + 1] = chunk_base[i] + Q * cs * t_cap

    def col0_of(w, q):
        ch = w // CW
        wl = w - ch * CW
        return chunk_base[ch] + q * csizes[ch] * t_cap + wl * t_cap

    for c, core in enumerate(cores):
        counts = core["counts"]
        starts = np.zeros(nt * Q + 1, np.int64)
        np.cumsum(counts, out=starts[1:])
        idxf = np.zeros(nslot, np.int16)
        dstff = np.full(nslot, -1.0, np.float32)
        ewtf = np.zeros(nslot, np.float32)
        srel, drel, wgt = core["srel"], core["drel"], core["wgt"]
        for w in range(nt):
            for q in range(Q):
                r = w * Q + q
                e0, e1 = starts[r], starts[r + 1]
                ln = e1 - e0
                o = col0_of(w, q) * P
                idxf[o:o + ln] = srel[e0:e1]
                dstff[o:o + ln] = drel[e0:e1]
                ewtf[o:o + ln] = wgt[e0:e1]
                if ln < t_cap * P:
                    pad_idx = srel[e1 - 1] if ln > 0 else 0
                    idxf[o + ln:o + t_cap * P] = pad_idx
        wrapped = idxf.reshape(-1, 16).T  # [16, nslot/16]
        idx16[c] = np.tile(wrapped, (8, 1))
        dstf[c] = dstff.reshape(ncols, P).T.astype(bfl)
        ewt[c] = ewtf.reshape(ncols, P).T.astype(bfl)
        dsttr[c] = dstff.reshape(ncols, P).astype(bfl)
    return idx16, dstf, ewt, dsttr, ncols


# ----------------------------------------------------------------------------
# Device program
# ----------------------------------------------------------------------------

def _build_program(n, f_in, f, f_out, nt, t_cap, consts, stop_after=None,
                   debug_dump=False, reps=1, phase_filter=None):
    import concourse.bass as bass
    import concourse.mybir as mybir
    import concourse.tile as tile
    import concourse.bacc as bacc

    fp32 = mybir.dt.float32
    bf16 = mybir.dt.bfloat16
    i16 = mybir.dt.int16
    AF = mybir.ActivationFunctionType
    OP = mybir.AluOpType
    ds = bass.ds
    nloc_pad = nt * P
    ngrows = NCORES * nloc_pad
    sub = ngrows // Q
    C = f // H
    csizes = _chunk_sizes(nt)
    ncols = Q * nt * t_cap
    QT = Q * t_cap

    nc = bacc.Bacc("TRN2", target_bir_lowering=False, debug=False,
                   num_devices=NCORES, num_swdge_queues=NQUEUE)

    # ---- I/O ----
    ext = {}
    for b in (1, 2):
        ext[f"xloc{b}"] = nc.dram_tensor(f"xloc{b}", [nloc_pad, f_in], fp32, kind="ExternalInput")
        ext[f"idx{b}"] = nc.dram_tensor(f"idx{b}", [128, ncols * 8], i16, kind="ExternalInput")
        ext[f"dstf{b}"] = nc.dram_tensor(f"dstf{b}", [P, ncols], bf16, kind="ExternalInput")
        ext[f"ewt{b}"] = nc.dram_tensor(f"ewt{b}", [P, ncols], bf16, kind="ExternalInput")
        ext[f"dsttr{b}"] = nc.dram_tensor(f"dsttr{b}", [ncols, P], bf16, kind="ExternalInput")
        ext[f"batf{b}"] = nc.dram_tensor(f"batf{b}", [P, nt], fp32, kind="ExternalInput")
        ext[f"batr{b}"] = nc.dram_tensor(f"batr{b}", [nt, P], bf16, kind="ExternalInput")
        ext[f"nself{b}"] = nc.dram_tensor(f"nself{b}", [P, nt], bf16, kind="ExternalInput")
        ext[f"out{b}"] = nc.dram_tensor(f"out{b}", [nloc_pad, f_out], fp32, kind="ExternalOutput")

    # ---- internal DRAM ----
    dram = {}
    for b in (1, 2):
        dram[f"hfull{b}"] = nc.dram_tensor(f"hfull{b}", [ngrows, f], bf16, addr_space="Shared")
        dram[f"bounce{b}"] = nc.dram_tensor(f"bounce{b}", [nloc_pad, f], bf16)
        dram[f"adt{b}"] = nc.dram_tensor(f"adt{b}", [nloc_pad, 2 * H], fp32)
        dram[f"agge{b}"] = nc.dram_tensor(f"agge{b}", [nloc_pad, f + H], fp32)
        dram[f"x{b}"] = nc.dram_tensor(f"x{b}", [nloc_pad, f], fp32)
        dram[f"gnh{b}"] = nc.dram_tensor(f"gnh{b}", [nloc_pad, f], bf16)
        for l in range(3):
            dram[f"arin{b}{l}"] = nc.dram_tensor(f"arin{b}{l}", [P, 2 * f], fp32)
            dram[f"arout{b}{l}"] = nc.dram_tensor(f"arout{b}{l}", [P, 2 * f], fp32, addr_space="Shared")

    cd = {k: nc.inline_tensor(v, name=f"c_{k}") for k, v in consts.items()}

    rg = [list(range(NCORES))]

    # node-phase blocks
    blocks = []
    t0b = 0
    while t0b < nt:
        blocks.append((t0b, min(TB, nt - t0b)))
        t0b += TB

    with tile.TileContext(nc) as tc:
        import contextlib
        stack = contextlib.ExitStack()
        sb_const = stack.enter_context(tc.tile_pool(name="const", bufs=1))
        sb_big = stack.enter_context(tc.tile_pool(name="big", bufs=1))
        sb_gth = stack.enter_context(tc.tile_pool(name="gth", bufs=2))
        sb_cat = stack.enter_context(tc.tile_pool(name="cat", bufs=1))
        sb_idx = stack.enter_context(tc.tile_pool(name="idx", bufs=2))
        sb_meta = stack.enter_context(tc.tile_pool(name="meta", bufs=2))
        sb_win = stack.enter_context(tc.tile_pool(name="win", bufs=2))
        sb_small = stack.enter_context(tc.tile_pool(name="small", bufs=2))
        sb_nblk = stack.enter_context(tc.tile_pool(name="nblk", bufs=2))
        sb_mid = stack.enter_context(tc.tile_pool(name="mid", bufs=1))
        ps_win = stack.enter_context(tc.tile_pool(name="pwin", bufs=2, space="PSUM"))
        ps_pade = stack.enter_context(tc.tile_pool(name="ppade", bufs=2, space="PSUM"))
        ps_bc = stack.enter_context(tc.tile_pool(name="pbc", bufs=2, space="PSUM"))
        ps_stats = stack.enter_context(tc.tile_pool(name="pstats", bufs=1, space="PSUM"))

        # ---- consts to SBUF ----
        cs = {}
        for k, t in cd.items():
            cs[k] = sb_const.tile(list(t.shape), t.dtype, name=f"s_{k}")
            nc.sync.dma_start(out=cs[k][:], in_=t[:])

        # ---- resident SBUF ----
        bat_sb = {}
        nself_sb = {}
        for b in (1, 2):
            bat_sb[b] = sb_big.tile([P, nt], fp32, name=f"bat_sb{b}")
            nc.sync.dma_start(out=bat_sb[b][:], in_=ext[f"batf{b}"][:])
            nself_sb[b] = sb_big.tile([P, nt], bf16, name=f"nself_sb{b}")
            nc.sync.dma_start(out=nself_sb[b][:], in_=ext[f"nself{b}"][:])
        stats_sb = sb_big.tile([P, 2 * f], fp32, name="stats_sb")
        nstats_sb = sb_big.tile([P, 2 * f], fp32, name="nstats_sb")

        def node_rows(dram_t, t0, tb, width):
            v = dram_t[:].rearrange("(t p) f -> p t f", p=P)[:, ds(t0, tb), :width]
            return v

        # ---------------- node phase: initial h0 = x @ W0^T -> bounce -------
        def phase_n0(b):
            for t0, tb in blocks:
                xst = sb_nblk.tile([P, tb, f_in], fp32, tag="num")
                nc.sync.dma_start(out=xst[:], in_=node_rows(ext[f"xloc{b}"], t0, tb, f_in))
                hb = sb_nblk.tile([P, tb, f], bf16, tag="hgel")
                for tl in range(tb):
                    pxt = ps_pade.tile([P, f_in], fp32, tag="pt")
                    nc.tensor.transpose(out=pxt[:], in_=xst[:][:, tl, :], identity=cs["ident"][:])
                    xT = sb_small.tile([P, f_in], fp32, tag="xT")
                    nc.vector.tensor_copy(out=xT[:], in_=pxt[:])
                    pha = ps_bc.tile([P, f], fp32, tag="bc")
                    nc.tensor.matmul(out=pha[:], lhsT=xT[:], rhs=cs["mov0"][:],
                                     start=True, stop=True, skip_group_check=True)
                    nc.scalar.copy(out=hb[:][:, tl, :], in_=pha[:])
                nc.sync.dma_start(out=node_rows(dram[f"bounce{b}"], t0, tb, f), in_=hb[:])

        # ---------------- edge phase ----------------
        def phase_edge(b, layer):
            gat = layer > 0
            col_base = 0
            gq = 0
            for ch, cwc in enumerate(csizes):
                ccols = Q * cwc * t_cap
                nslots = ccols * P
                QTl = Q * t_cap
                idxt = sb_idx.tile([P, nslots // 16], i16, tag="idxt")
                nc.sync.dma_start(out=idxt[:], in_=ext[f"idx{b}"][:, col_base * 8:col_base * 8 + nslots // 16])
                dstft = sb_meta.tile([P, ccols], bf16, tag="dstft")
                nc.sync.dma_start(out=dstft[:], in_=ext[f"dstf{b}"][:, col_base:col_base + ccols])
                if not gat:
                    ewtt = sb_meta.tile([P, ccols], bf16, tag="ewtt")
                    nc.sync.dma_start(out=ewtt[:], in_=ext[f"ewt{b}"][:, col_base:col_base + ccols])
                gth = sb_gth.tile([P, ccols, f], bf16, tag="gth")
                if "nogather" not in _ABL:
                    for q in range(Q):
                        qoff = q * cwc * t_cap * P
                        qslots = cwc * t_cap * P
                        o = 0
                        while o < qslots:
                            nidx = min(GCALL, qslots - o)
                            nc.gpsimd.dma_gather(
                                gth[:][:, (qoff + o) // P:(qoff + o + nidx) // P, :],
                                dram[f"hfull{b}"][ds(q * sub, sub), :],
                                idxt[:][:, (qoff + o) // 16:(qoff + o + nidx) // 16],
                                nidx, nidx, f, queue_num=gq % NQUEUE)
                            gq += 1
                            o += nidx
                if "gatheronly" in _ABL:
                    col_base += ccols
                    continue

                def dstf_w(iw):
                    # [P, Q, t_cap, P] broadcast view of dstft cols of window iw
                    v = dstft[:].rearrange("p (q w t) -> p q w t", q=Q, w=cwc)
                    v = v[:, :, ds(iw, 1), :].rearrange("p q o t -> p q (o t)")
                    v = v.rearrange("p q (t o) -> p q t o", o=1)
                    return v.broadcast_to([P, Q, t_cap, P])

                def iotab_w():
                    v = cs["iotab"][:].rearrange("p (a b e) -> p a b e", a=1, b=1)
                    return v.broadcast_to([P, Q, t_cap, P])

                stage = sb_small.tile([P, cwc, f + H], fp32, tag="stage")
                if gat:
                    # window-major [P, cwc, QTl*H] attention buffers
                    eatt = sb_small.tile([P, cwc, QTl * H], fp32, tag="eatt")

                    def gth_w(iw):
                        # [P, Q, t_cap, f] view of gth at window iw
                        v = gth[:].rearrange("p (q w t) c -> p q w t c", q=Q, w=cwc)
                        return v[:, :, ds(iw, 1), :, :].rearrange("p q o t c -> p q (o t) c")

                    # sweep 1: per-window a_s + a_d -> eatt.  The eatt add for
                    # window iw is emitted one window late so the in-order DVE
                    # stream does not stall waiting for iw's pade matmuls.
                    pend = []

                    def flush_eatt():
                        iwp, padep, asrwp = pend.pop(0)
                        nc.vector.tensor_tensor(
                            out=eatt[:][:, ds(iwp, 1), :]
                                .rearrange("p o (k h) -> p (o k) h", h=H),
                            in0=padep[:],
                            in1=asrwp[:].rearrange("p (k h) -> p k h", h=H),
                            op=OP.add)

                    for iw in range(cwc):
                        w = ch * CW + iw
                        t0w = sb_win.tile([P, QTl, f], bf16, tag="t0w")
                        nc.vector.tensor_tensor(
                            out=t0w[:].rearrange("p (q t) c -> p q t c", q=Q),
                            in0=gth_w(iw),
                            in1=cs[f"attrep{layer}"][:]
                                .rearrange("p (o e) -> p o e", o=1)
                                .rearrange("p o (s e) -> p o s e", s=1)
                                .broadcast_to([P, Q, t_cap, f]),
                            op=OP.mult)
                        asrw = sb_small.tile([P, QTl * H], fp32, tag="asrw")
                        nc.vector.tensor_reduce(
                            out=asrw[:].rearrange("p (a h) -> p a h", h=H),
                            in_=t0w[:].rearrange("p a (h c) -> p a h c", c=C),
                            axis=mybir.AxisListType.X, op=OP.add)
                        dstb = sb_win.tile([P, QTl, P], bf16, tag="dstb")
                        sel = ext[f"dsttr{b}"][ds(col_base, ccols), :]
                        sel = sel.rearrange("(q w t) e -> q w t e", q=Q, w=cwc)
                        sel = sel[:, ds(iw, 1), :, :].rearrange("q o t e -> o q t e")
                        nc.sync.dma_start(
                            out=dstb[:].rearrange("p (q t) e -> p q t e", q=Q),
                            in_=sel.broadcast_to([P, Q, t_cap, P]))
                        ST = sb_win.tile([P, QTl, P], bf16, tag="ST")
                        nc.vector.tensor_tensor(
                            out=ST[:], in0=dstb[:],
                            in1=cs["iotacb"][:].rearrange("p (o e) -> p o e", o=1)
                                .broadcast_to([P, QTl, P]),
                            op=OP.is_equal)
                        adb = sb_small.tile([P, 2 * H], fp32, tag="adb")
                        nc.sync.dma_start(out=adb[:],
                                          in_=node_rows(dram[f"adt{b}"], w, 1, 2 * H)
                                          .rearrange("p o e -> p (o e)"))
                        adbb = sb_small.tile([P, H], bf16, tag="adbb")
                        nc.scalar.copy(out=adbb[:], in_=adb[:, :H])
                        pade = ps_pade.tile([P, QTl, H], fp32, tag="pt")
                        for k in range(QTl):
                            nc.tensor.matmul(out=pade[:][:, k, :], lhsT=ST[:][:, k, :],
                                             rhs=adbb[:], start=True, stop=True,
                                             skip_group_check=True)
                        pend.append((iw, pade, asrw))
                        if len(pend) > 1:
                            flush_eatt()
                    while pend:
                        flush_eatt()
                    # chunk-wide leaky relu + exp
                    lr = sb_small.tile([P, ccols * H], fp32, tag="lr")
                    eflat = eatt[:].rearrange("p w e -> p (w e)")
                    nc.vector.tensor_scalar_mul(out=lr[:], in0=eflat, scalar1=0.2)
                    nc.vector.tensor_max(out=lr[:], in0=lr[:], in1=eflat)
                    exch = sb_small.tile([P, cwc, QTl * H], bf16, tag="exch")
                    nc.scalar.activation(
                        out=exch[:].rearrange("p w e -> p (w e)"),
                        in_=lr[:], func=AF.Exp)
                    for iw in range(cwc):
                        # [alpha-weighted features | exp] for this window
                        alphw = sb_win.tile([P, QTl, f + H], bf16, tag="alphw")
                        for q in range(Q):
                            colq0 = (q * cwc + iw) * t_cap
                            nc.vector.tensor_tensor(
                                out=alphw[:][:, ds(q * t_cap, t_cap), :f]
                                    .rearrange("p t (h c) -> p t h c", c=C),
                                in0=gth[:][:, ds(colq0, t_cap), :]
                                    .rearrange("p t (h c) -> p t h c", c=C),
                                in1=exch[:][:, ds(iw, 1), ds(q * t_cap * H, t_cap * H)]
                                    .rearrange("p o (t h) -> p (o t) h", h=H)
                                    .rearrange("p t (h u) -> p t h u", u=1)
                                    .broadcast_to([P, t_cap, H, C]),
                                op=OP.mult)
                        nc.vector.tensor_copy(
                            out=alphw[:][:, :, f:],
                            in_=exch[:][:, ds(iw, 1), :]
                                .rearrange("p o (t h) -> p (o t) h", h=H))
                        S = sb_win.tile([P, QTl, P], bf16, tag="S")
                        nc.vector.tensor_tensor(
                            out=S[:].rearrange("p (q t) e -> p q t e", q=Q),
                            in0=dstf_w(iw), in1=iotab_w(), op=OP.is_equal)
                        pwin = ps_win.tile([P, f + H], fp32, tag="pwin")
                        for k in range(QTl):
                            nc.tensor.matmul(out=pwin[:], lhsT=S[:][:, k, :],
                                             rhs=alphw[:][:, k, :],
                                             start=(k == 0), stop=(k == QTl - 1),
                                             skip_group_check=True)
                        nc.scalar.copy(out=stage[:][:, iw, :], in_=pwin[:])
                else:
                    for iw in range(cwc):
                        S = sb_win.tile([P, QTl, P], bf16, tag="S")
                        nc.vector.tensor_tensor(
                            out=S[:].rearrange("p (q t) e -> p q t e", q=Q),
                            in0=dstf_w(iw), in1=iotab_w(), op=OP.is_equal)
                        # fold GCN edge weights into the one-hot
                        v = ewtt[:].rearrange("p (q w t) -> p q w t", q=Q, w=cwc)
                        v = v[:, :, ds(iw, 1), :].rearrange("p q o t -> p q (o t)")
                        v = v.rearrange("p q (t o) -> p q t o", o=1)
                        nc.vector.tensor_tensor(
                            out=S[:].rearrange("p (q t) e -> p q t e", q=Q),
                            in0=S[:].rearrange("p (q t) e -> p q t e", q=Q),
                            in1=v.broadcast_to([P, Q, t_cap, P]), op=OP.mult)
                        pwin = ps_win.tile([P, f], fp32, tag="pwin")
                        for k in range(QTl):
                            col = (k // t_cap * cwc + iw) * t_cap + (k % t_cap)
                            nc.tensor.matmul(out=pwin[:], lhsT=S[:][:, k, :],
                                             rhs=gth[:][:, col, :],
                                             start=(k == 0), stop=(k == QTl - 1),
                                             skip_group_check=True)
                        nc.scalar.copy(out=stage[:][:, iw, :f], in_=pwin[:])
                nc.sync.dma_start(
                    out=node_rows(dram[f"agge{b}"], ch * CW, cwc, f + H),
                    in_=stage[:])
                col_base += ccols

        # ---------------- node phase (A: stats + AllReduce issue) ----------
        def phase_node_a(b, layer):
            gat = layer > 0
            if gat:
                # self-loop attention: one sweep, single Exp
                adts = sb_mid.tile([P, nt, 2 * H], fp32, tag="adts")
                nc.sync.dma_start(out=adts[:], in_=node_rows(dram[f"adt{b}"], 0, nt, 2 * H))
                exf = sb_mid.tile([P, nt * H], fp32, tag="exf")
                lrn = sb_mid.tile([P, nt * H], fp32, tag="lrn")
                nc.vector.tensor_tensor(out=lrn[:].rearrange("p (t h) -> p t h", h=H),
                                        in0=adts[:][:, :, :H], in1=adts[:][:, :, H:],
                                        op=OP.add)
                nc.vector.tensor_scalar_mul(out=exf[:], in0=lrn[:], scalar1=0.2)
                nc.vector.tensor_max(out=lrn[:], in0=lrn[:], in1=exf[:])
                nc.scalar.activation(out=exf[:], in_=lrn[:], func=AF.Exp)
                exfb = sb_mid.tile([P, nt * H], bf16, tag="exfb")
                nc.vector.tensor_copy(out=exfb[:], in_=exf[:])

            # pass A: combine self+edge aggregation, per-graph moments
            pstat1 = ps_stats.tile([P, f], fp32, tag="st1")
            pstat2 = ps_stats.tile([P, f], fp32, tag="st2")
            for bi, (t0, tb) in enumerate(blocks):
                hs = sb_nblk.tile([P, tb, f], bf16, tag="hs")
                nc.sync.dma_start(out=hs[:], in_=node_rows(dram[f"bounce{b}"], t0, tb, f))
                num = sb_nblk.tile([P, tb, f], fp32, tag="num")
                if gat:
                    ai = sb_nblk.tile([P, tb, f + H], fp32, tag="ai")
                    nc.sync.dma_start(out=ai[:], in_=node_rows(dram[f"agge{b}"], t0, tb, f + H))
                    exs = exf[:].rearrange("p (t h) -> p t h", h=H)[:, ds(t0, tb), :]
                    exsb = exfb[:].rearrange("p (t h) -> p t h", h=H)[:, ds(t0, tb), :]
                    nc.vector.tensor_tensor(
                        out=num[:].rearrange("p t (h c) -> p t h c", c=C),
                        in0=hs[:].rearrange("p t (h c) -> p t h c", c=C),
                        in1=exsb.rearrange("p t (h o) -> p t h o", o=1)
                            .broadcast_to([P, tb, H, C]),
                        op=OP.mult)
                    nc.vector.tensor_tensor(out=num[:], in0=num[:], in1=ai[:][:, :, :f],
                                            op=OP.add)
                    den = sb_nblk.tile([P, tb * H], fp32, tag="den")
                    nc.vector.tensor_tensor(out=den[:].rearrange("p (t h) -> p t h", h=H),
                                            in0=ai[:][:, :, f:], in1=exs, op=OP.add)
                    nc.vector.tensor_scalar_add(out=den[:], in0=den[:], scalar1=1e-16)
                    nc.vector.reciprocal(out=den[:], in_=den[:])
                    nc.vector.tensor_tensor(
                        out=num[:].rearrange("p t (h c) -> p t h c", c=C),
                        in0=num[:].rearrange("p t (h c) -> p t h c", c=C),
                        in1=den[:].rearrange("p (t h o) -> p t h o", h=H, o=1)
                            .broadcast_to([P, tb, H, C]),
                        op=OP.mult)
                    nc.vector.tensor_tensor(
                        out=num[:], in0=num[:],
                        in1=cs[f"brow{layer}"][:].rearrange("p (o e) -> p o e", o=1)
                            .broadcast_to([P, tb, f]),
                        op=OP.add)
                    hgel = sb_nblk.tile([P, tb, f], bf16, tag="hgel")
                    nc.scalar.activation(out=hgel[:], in_=num[:], func=AF.Gelu)
                else:
                    ai = sb_nblk.tile([P, tb, f], fp32, tag="ai0")
                    nc.sync.dma_start(out=ai[:], in_=node_rows(dram[f"agge{b}"], t0, tb, f))
                    nc.vector.tensor_tensor(
                        out=num[:], in0=hs[:],
                        in1=nself_sb[b][:, ds(t0, tb)].rearrange("p (t o) -> p t o", o=1)
                            .broadcast_to([P, tb, f]),
                        op=OP.mult)
                    nc.vector.tensor_tensor(out=num[:], in0=num[:], in1=ai[:], op=OP.add)
                    nc.vector.tensor_tensor(
                        out=num[:], in0=num[:],
                        in1=cs["brow0"][:].rearrange("p (o e) -> p o e", o=1)
                            .broadcast_to([P, tb, f]),
                        op=OP.add)
                    hgel = sb_nblk.tile([P, tb, f], bf16, tag="hgel")
                    nc.vector.tensor_copy(out=hgel[:], in_=num[:])
                nc.sync.dma_start(out=node_rows(dram[f"gnh{b}"], t0, tb, f), in_=hgel[:])
                sq = sb_nblk.tile([P, tb, f], bf16, tag="sq")
                nc.scalar.activation(out=sq[:], in_=hgel[:], func=AF.Square)
                Bblk = sb_nblk.tile([P, tb, P], bf16, tag="Bblk")
                nc.vector.tensor_tensor(
                    out=Bblk[:],
                    in0=bat_sb[b][:, ds(t0, tb)].rearrange("p (t o) -> p t o", o=1)
                        .broadcast_to([P, tb, P]),
                    in1=cs["iota128"][:].rearrange("p (o e) -> p o e", o=1)
                        .broadcast_to([P, tb, P]),
                    op=OP.is_equal)
                for tl in range(tb):
                    t = t0 + tl
                    nc.tensor.matmul(out=pstat1[:], lhsT=Bblk[:][:, tl, :],
                                     rhs=hgel[:][:, tl, :],
                                     start=(t == 0), stop=(t == nt - 1),
                                     skip_group_check=True)
                    nc.tensor.matmul(out=pstat2[:], lhsT=Bblk[:][:, tl, :],
                                     rhs=sq[:][:, tl, :],
                                     start=(t == 0), stop=(t == nt - 1),
                                     skip_group_check=True)
            nc.vector.tensor_copy(out=stats_sb[:, :f], in_=pstat1[:])
            nc.vector.tensor_copy(out=stats_sb[:, f:], in_=pstat2[:])
            # AllReduce the moments
            nc.sync.dma_start(out=dram[f"arin{b}{layer}"][:], in_=stats_sb[:])
            nc.gpsimd.collective_compute(
                "AllReduce", mybir.AluOpType.add, replica_groups=rg,
                ins=[dram[f"arin{b}{layer}"][:]], outs=[dram[f"arout{b}{layer}"][:]])

        # ---------------- node phase (B: normalize + next matmul) ----------
        def phase_node_b(b, layer):
            gat = layer > 0
            lname = ["gn0", "gn1", "gn2"][layer]
            nc.sync.dma_start(out=nstats_sb[:], in_=dram[f"arout{b}{layer}"][:])
            # per-graph coefficients: out = x*A[g] - Bv[g]
            mean = sb_mid.tile([P, f], fp32, tag="mean")
            nc.vector.tensor_scalar_mul(out=mean[:], in0=nstats_sb[:, :f],
                                        scalar1=cs[f"cntinv{b}"][:, ds(0, 1)])
            e2 = sb_mid.tile([P, f], fp32, tag="e2")
            nc.vector.tensor_scalar_mul(out=e2[:], in0=nstats_sb[:, f:],
                                        scalar1=cs[f"cntinv{b}"][:, ds(0, 1)])
            m2 = sb_mid.tile([P, f], fp32, tag="m2")
            nc.vector.tensor_mul(out=m2[:], in0=mean[:], in1=mean[:])
            nc.vector.tensor_mul(out=m2[:], in0=m2[:], in1=cs[f"am_{lname}"][:])
            nc.vector.tensor_sub(out=e2[:], in0=e2[:], in1=m2[:])
            nc.vector.tensor_scalar_add(out=e2[:], in0=e2[:], scalar1=EPS)
            sd = sb_mid.tile([P, f], fp32, tag="sd")
            nc.scalar.activation(out=sd[:], in_=e2[:], func=AF.Sqrt)
            normAB = sb_mid.tile([P, 2 * f], fp32, tag="normAB")
            nc.vector.reciprocal(out=normAB[:, f:], in_=sd[:])
            nc.vector.tensor_mul(out=normAB[:, f:], in0=normAB[:, f:],
                                 in1=cs[f"ga_{lname}"][:])
            nc.vector.tensor_mul(out=normAB[:, :f], in0=mean[:], in1=cs[f"al_{lname}"][:])
            nc.vector.tensor_mul(out=normAB[:, :f], in0=normAB[:, :f], in1=normAB[:, f:])
            nc.vector.tensor_sub(out=normAB[:, :f], in0=normAB[:, :f],
                                 in1=cs[f"be_{lname}"][:])
            normABb = sb_mid.tile([P, 2 * f], bf16, tag="normABb")
            nc.vector.tensor_copy(out=normABb[:], in_=normAB[:])
            # pass B: normalize, residual, next-layer matmul
            for t0, tb in blocks:
                batrb = sb_nblk.tile([P, tb, P], bf16, tag="batrb")
                selb = ext[f"batr{b}"][ds(t0, tb), :].rearrange("(o t) e -> o t e", o=1)
                nc.sync.dma_start(out=batrb[:], in_=selb.broadcast_to([P, tb, P]))
                BTb = sb_nblk.tile([P, tb, P], bf16, tag="BTb")
                nc.vector.tensor_tensor(
                    out=BTb[:], in0=batrb[:],
                    in1=cs["iotacb"][:].rearrange("p (o e) -> p o e", o=1)
                        .broadcast_to([P, tb, P]),
                    op=OP.is_equal)
                hgb = sb_nblk.tile([P, tb, f], bf16, tag="hgb")
                nc.sync.dma_start(out=hgb[:], in_=node_rows(dram[f"gnh{b}"], t0, tb, f))
                hgb32 = sb_nblk.tile([P, tb, f], fp32, tag="num")
                nc.vector.tensor_copy(out=hgb32[:], in_=hgb[:])
                xnb = sb_nblk.tile([P, tb, f], fp32, tag="xnb")
                for tl in range(tb):
                    pbc = ps_bc.tile([P, 2 * f], fp32, tag="bc")
                    nc.tensor.matmul(out=pbc[:], lhsT=BTb[:][:, tl, :], rhs=normABb[:],
                                     start=True, stop=True, skip_group_check=True)
                    nc.vector.tensor_tensor(out=xnb[:][:, tl, :], in0=hgb32[:][:, tl, :],
                                            in1=pbc[:, f:], op=OP.mult)
                    nc.vector.tensor_tensor(out=xnb[:][:, tl, :], in0=xnb[:][:, tl, :],
                                            in1=pbc[:, :f], op=OP.subtract)
                if gat:
                    xpb = sb_nblk.tile([P, tb, f], fp32, tag="xpb")
                    nc.sync.dma_start(out=xpb[:], in_=node_rows(dram[f"x{b}"], t0, tb, f))
                    nc.vector.tensor_tensor(out=xnb[:], in0=xnb[:], in1=xpb[:], op=OP.add)
                if layer < 2:
                    nc.sync.dma_start(out=node_rows(dram[f"x{b}"], t0, tb, f), in_=xnb[:])
                    hbb = sb_nblk.tile([P, tb, f], bf16, tag="hbb")
                    adbk = sb_nblk.tile([P, tb, 2 * H], fp32, tag="adbk")
                else:
                    obb = sb_nblk.tile([P, tb, f_out], fp32, tag="obb")
                for tl in range(tb):
                    pxt = ps_pade.tile([P, f], fp32, tag="pt")
                    nc.tensor.transpose(out=pxt[:], in_=xnb[:][:, tl, :],
                                        identity=cs["ident"][:])
                    xT = sb_small.tile([P, f], fp32, tag="xT")
                    nc.vector.tensor_copy(out=xT[:], in_=pxt[:])
                    if layer < 2:
                        pha = ps_bc.tile([P, f + 2 * H], fp32, tag="bc")
                        nc.tensor.matmul(out=pha[:], lhsT=xT[:], rhs=cs[f"mov{layer + 1}"][:],
                                         start=True, stop=True, skip_group_check=True)
                        nc.scalar.copy(out=hbb[:][:, tl, :], in_=pha[:, :f])
                        nc.vector.tensor_copy(out=adbk[:][:, tl, :], in_=pha[:, f:])
                    else:
                        pha = ps_bc.tile([P, f_out], fp32, tag="bc")
                        nc.tensor.matmul(out=pha[:], lhsT=xT[:], rhs=cs["movL"][:],
                                         start=True, stop=True, skip_group_check=True)
                        nc.vector.tensor_tensor(out=obb[:][:, tl, :], in0=pha[:],
                                                in1=cs["linb"][:], op=OP.add)
                if layer < 2:
                    nc.sync.dma_start(out=node_rows(dram[f"bounce{b}"], t0, tb, f), in_=hbb[:])
                    nc.sync.dma_start(out=node_rows(dram[f"adt{b}"], t0, tb, 2 * H), in_=adbk[:])
                else:
                    nc.sync.dma_start(out=node_rows(ext[f"out{b}"], t0, tb, f_out), in_=obb[:])

        def ag(b):
            nc.gpsimd.collective_compute(
                "AllGather", mybir.AluOpType.bypass, replica_groups=rg,
                ins=[dram[f"bounce{b}"][:]], outs=[dram[f"hfull{b}"][:]])

        # ---------------- schedule ----------------
        # Interleaved so each branch's AllReduce (between nA and nB) hides
        # under the other branch's edge phase.
        def pe(b, l):
            return lambda: phase_edge(b, l)

        def pa(b, l):
            return lambda: phase_node_a(b, l)

        def pb(b, l):
            return lambda: phase_node_b(b, l)

        if os.environ.get("SCHED") == "seq":
            sched = [
                ("n0_1", lambda: phase_n0(1)), ("ag1a", lambda: ag(1)),
                ("n0_2", lambda: phase_n0(2)), ("ag2a", lambda: ag(2)),
                ("edge1_0", pe(1, 0)), ("nA1_0", pa(1, 0)), ("nB1_0", pb(1, 0)), ("ag1b", lambda: ag(1)),
                ("edge2_0", pe(2, 0)), ("nA2_0", pa(2, 0)), ("nB2_0", pb(2, 0)), ("ag2b", lambda: ag(2)),
                ("edge1_1", pe(1, 1)), ("nA1_1", pa(1, 1)), ("nB1_1", pb(1, 1)), ("ag1c", lambda: ag(1)),
                ("edge2_1", pe(2, 1)), ("nA2_1", pa(2, 1)), ("nB2_1", pb(2, 1)), ("ag2c", lambda: ag(2)),
                ("edge1_2", pe(1, 2)), ("nA1_2", pa(1, 2)), ("nB1_2", pb(1, 2)),
                ("edge2_2", pe(2, 2)), ("nA2_2", pa(2, 2)), ("nB2_2", pb(2, 2)),
            ]
        else:
            sched = [
                ("n0_1", lambda: phase_n0(1)), ("ag1a", lambda: ag(1)),
                ("n0_2", lambda: phase_n0(2)), ("ag2a", lambda: ag(2)),
                ("edge1_0", pe(1, 0)), ("nA1_0", pa(1, 0)),
                ("edge2_0", pe(2, 0)), ("nB1_0", pb(1, 0)), ("ag1b", lambda: ag(1)),
                ("nA2_0", pa(2, 0)),
                ("edge1_1", pe(1, 1)), ("nB2_0", pb(2, 0)), ("ag2b", lambda: ag(2)),
                ("nA1_1", pa(1, 1)),
                ("edge2_1", pe(2, 1)), ("nB1_1", pb(1, 1)), ("ag1c", lambda: ag(1)),
                ("nA2_1", pa(2, 1)),
                ("edge1_2", pe(1, 2)), ("nB2_1", pb(2, 1)), ("ag2c", lambda: ag(2)),
                ("nA1_2", pa(1, 2)),
                ("edge2_2", pe(2, 2)), ("nB1_2", pb(1, 2)),
                ("nA2_2", pa(2, 2)), ("nB2_2", pb(2, 2)),
            ]
        for rep in range(reps):
            for i, (pname, ph) in enumerate(sched):
                if stop_after is not None and i >= stop_after:
                    break
                if phase_filter is not None and not any(
                        pname.startswith(p) for p in phase_filter):
                    continue
                with nc.named_scope(f"{pname}_r{rep}" if rep else pname):
                    ph()
        if debug_dump:
            shapes = {"agge1": [nloc_pad, f + H], "adt1": [nloc_pad, 2 * H],
                      "bounce1": [nloc_pad, f], "x1": [nloc_pad, f],
                      "gnh1": [nloc_pad, f],
                      "hfull1": [ngrows, f]}
            names = debug_dump if isinstance(debug_dump, (list, tuple)) else list(shapes)
            for snm in names:
                if snm == "ar10":
                    t = nc.dram_tensor("d_ar10", [P, 2 * f], fp32, kind="ExternalOutput")
                    nc.sync.dma_start(out=t[:], in_=dram["arout10"][:])
                    continue
                src_t = dram[snm]
                t = nc.dram_tensor(f"d_{snm}", shapes[snm], src_t.dtype, kind="ExternalOutput")
                nc.sync.dma_start(out=t[:], in_=src_t[:])
        stack.close()

    nc.compile()
    return nc


# ----------------------------------------------------------------------------
# Const construction
# ----------------------------------------------------------------------------

def _make_consts(f, f_out, W0, b0, gn0_gamma, gn0_beta, gn0_alpha, gat_W,
                 gat_att_src, gat_att_dst, gat_b, gn_gamma, gn_beta, gn_alpha,
                 lin_W, lin_b, cnt1, cnt2):
    C = f // H
    cons = {}
    cons["ident"] = np.eye(P, dtype=np.float32)
    cons["iota128"] = np.tile(np.arange(P, dtype=np.float32), (P, 1))
    cons["iotab"] = np.tile(np.arange(P, dtype=np.float32), (P, 1)).astype(bfl)
    cons["iotacb"] = np.tile(np.arange(P, dtype=np.float32)[:, None], (1, P)).astype(bfl)

    cons["mov0"] = W0.T.astype(np.float32).copy()
    for l in range(2):
        Adm = np.zeros((f, H), np.float32)
        Asm = np.zeros((f, H), np.float32)
        for hh in range(H):
            Adm[hh * C:(hh + 1) * C, hh] = gat_att_dst[l][hh]
            Asm[hh * C:(hh + 1) * C, hh] = gat_att_src[l][hh]
        cons[f"mov{l + 1}"] = np.concatenate(
            [gat_W[l].T, gat_W[l].T @ Adm, gat_W[l].T @ Asm], axis=1).astype(np.float32)
        cons[f"brow{l + 1}"] = np.tile(gat_b[l], (P, 1)).astype(np.float32)
        cons[f"attrep{l + 1}"] = np.tile(gat_att_src[l].reshape(-1), (P, 1)).astype(bfl)
    cons["movL"] = lin_W.T.astype(np.float32).copy()
    cons["linb"] = np.tile(lin_b, (P, 1)).astype(np.float32)
    cons["brow0"] = np.tile(b0, (P, 1)).astype(np.float32)
    for lname, ga, be, al in [("gn0", gn0_gamma, gn0_beta, gn0_alpha),
                              ("gn1", gn_gamma[0], gn_beta[0], gn_alpha[0]),
                              ("gn2", gn_gamma[1], gn_beta[1], gn_alpha[1])]:
        cons[f"ga_{lname}"] = np.tile(ga, (P, 1)).astype(np.float32)
        cons[f"be_{lname}"] = np.tile(be, (P, 1)).astype(np.float32)
        cons[f"al_{lname}"] = np.tile(al, (P, 1)).astype(np.float32)
        cons[f"am_{lname}"] = np.tile(2 * al - al * al, (P, 1)).astype(np.float32)
    cons["cntinv1"] = np.concatenate([1.0 / cnt1, np.ones(P - G, np.float32)])[:, None].astype(np.float32)
    cons["cntinv2"] = np.concatenate([1.0 / cnt2, np.ones(P - G, np.float32)])[:, None].astype(np.float32)
    return cons


# ----------------------------------------------------------------------------
# Input packing per core
# ----------------------------------------------------------------------------

def make_in_maps(x1, x2, edge_index1, edge_index2, batch1, batch2, n):
    nloc = n // NCORES
    nt = (nloc + P - 1) // P
    nloc_pad = nt * P
    cores1, nself1, mr1 = _prep_branch(edge_index1, n, nloc, nloc_pad)
    cores2, nself2, mr2 = _prep_branch(edge_index2, n, nloc, nloc_pad)
    t_cap = (max(mr1, mr2) + P - 1) // P
    idx1, dstf1, ewt1, dsttr1, ncols = _pack_branch(cores1, nt, t_cap)
    idx2, dstf2, ewt2, dsttr2, _ = _pack_branch(cores2, nt, t_cap)

    in_maps = []
    for c in range(NCORES):
        m = {}
        for b, x, bat, nself, idx, dstf, ewt, dsttr in (
                (1, x1, batch1, nself1, idx1, dstf1, ewt1, dsttr1),
                (2, x2, batch2, nself2, idx2, dstf2, ewt2, dsttr2)):
            xl = np.zeros((nloc_pad, x.shape[1]), np.float32)
            xl[:nloc] = x[c * nloc:(c + 1) * nloc]
            bl = np.full(nloc_pad, -1.0, np.float32)
            bl[:nloc] = bat[c * nloc:(c + 1) * nloc].astype(np.float32)
            nsl = np.zeros(nloc_pad, np.float32)
            nsl[:nloc] = nself[c * nloc:(c + 1) * nloc]
            m[f"xloc{b}"] = xl
            m[f"batf{b}"] = bl.reshape(nt, P).T.copy()
            m[f"batr{b}"] = bl.reshape(nt, P).astype(bfl)
            m[f"nself{b}"] = nsl.reshape(nt, P).T.astype(bfl)
            m[f"idx{b}"] = idx[c]
            m[f"dstf{b}"] = dstf[c]
            m[f"ewt{b}"] = ewt[c]
            m[f"dsttr{b}"] = dsttr[c]
        in_maps.append(m)
    return in_maps, nt, t_cap, nloc, nloc_pad


# ----------------------------------------------------------------------------
# PJRT runner (reusable jitted executable)
# ----------------------------------------------------------------------------

class _Runner:
    def __init__(self, nc, n_cores):
        import jax
        from jax.sharding import Mesh, PartitionSpec
        from jax.experimental.shard_map import shard_map
        import concourse.mybir as mybir
        from concourse import bass2jax
        from concourse.bass2jax import _bass_exec_p, install_neuronx_cc_hook

        install_neuronx_cc_hook()
        self.jax = jax
        self.nc = nc
        self.n_cores = n_cores
        partition_name = (
            nc.partition_id_tensor.name if nc.partition_id_tensor else None)
        dbg_name = nc.dbg_addr.name if nc.dbg_addr else None
        in_names, out_names, out_avals, zero_outs = [], [], [], []
        for alloc in nc.m.functions[0].allocations:
            if not isinstance(alloc, mybir.MemoryLocationSet):
                continue
            name = alloc.memorylocations[0].name
            if alloc.kind == "ExternalInput":
                if name not in (partition_name, dbg_name):
                    in_names.append(name)
            elif alloc.kind == "ExternalOutput":
                out_names.append(name)
                shape = tuple(alloc.tensor_shape)
                dtype = mybir.dt.np(alloc.dtype)
                out_avals.append(jax.core.ShapedArray(shape, dtype))
                zero_outs.append(np.zeros(shape, dtype))
        self.in_names, self.out_names = in_names, out_names
        self.out_avals, self.zero_outs = out_avals, zero_outs
        n_params, n_outs = len(in_names), len(out_names)
        all_in_names = list(in_names) + list(out_names)
        if dbg_name is not None:
            all_in_names.append(dbg_name)
        if partition_name is not None:
            all_in_names.append(partition_name)

        def _body(*args):
            operands = list(args)
            if dbg_name is not None:
                operands.append(np.zeros((1, 2), np.uint32))
            if partition_name is not None:
                operands.append(bass2jax.partition_id_tensor())
            outs = _bass_exec_p.bind(
                *operands, out_avals=tuple(out_avals), in_names=tuple(all_in_names),
                out_names=tuple(out_names), lowering_input_output_aliases=(),
                sim_require_finite=False, sim_require_nnan=False, nc=nc)
            return tuple(outs)

        devices = jax.devices()[:n_cores]
        self.mesh = Mesh(np.asarray(devices), ("core",))
        in_specs = (PartitionSpec("core"),) * (n_params + n_outs)
        out_specs = (PartitionSpec("core"),) * n_outs
        self.fn = jax.jit(
            shard_map(_body, mesh=self.mesh, in_specs=in_specs,
                      out_specs=out_specs, check_rep=False),
            keep_unused=True)

    def stage(self, in_maps):
        import jax
        from jax.sharding import PartitionSpec
        n = self.n_cores
        arrs = [np.concatenate([np.asarray(in_maps[c][k]) for c in range(n)], axis=0)
                for k in self.in_names]
        arrs += [np.zeros((n * z.shape[0], *z.shape[1:]), z.dtype) for z in self.zero_outs]
        sh = jax.sharding.NamedSharding(self.mesh, PartitionSpec("core"))
        self._staged = [jax.device_put(a, sh) for a in arrs]

    def run(self):
        outs = self.fn(*self._staged)
        self.jax.block_until_ready(outs)
        return outs

    def results(self, outs):
        n = self.n_cores
        return [
            {name: np.asarray(outs[i]).reshape(n, *self.out_avals[i].shape)[c]
             for i, name in enumerate(self.out_names)}
            for c in range(n)]


# ----------------------------------------------------------------------------
# Entry point
# ----------------------------------------------------------------------------

def _get_runner_and_inmaps(x1, x2, edge_index1, edge_index2, batch1, batch2,
                           params, reps=1, stop_after=None, debug_dump=False,
                           phase_filter=None):
    n, f_in = x1.shape
    in_maps, nt, t_cap, nloc, nloc_pad = make_in_maps(
        x1, x2, edge_index1, edge_index2, batch1, batch2, n)
    cnt1 = np.maximum(np.bincount(batch1, minlength=G), 1).astype(np.float32)
    cnt2 = np.maximum(np.bincount(batch2, minlength=G), 1).astype(np.float32)
    f = params["W0"].shape[0]
    f_out = params["lin_W"].shape[0]
    consts = _make_consts(f, f_out, params["W0"], params["b0"],
                          params["gn0_gamma"], params["gn0_beta"], params["gn0_alpha"],
                          params["gat_W"], params["gat_att_src"], params["gat_att_dst"],
                          params["gat_b"], params["gn_gamma"], params["gn_beta"],
                          params["gn_alpha"], params["lin_W"], params["lin_b"],
                          cnt1, cnt2)
    nc = _build_program(n, f_in, f, f_out, nt, t_cap, consts, reps=reps,
                        stop_after=stop_after, debug_dump=debug_dump,
                        phase_filter=phase_filter)
    runner = _Runner(nc, NCORES)
    return runner, in_maps, nloc, nloc_pad, f_out


def kernel(x1, x2, edge_index1, edge_index2, batch1, batch2,
           W0, b0, gn0_gamma, gn0_beta, gn0_alpha,
           gat_W, gat_att_src, gat_att_dst, gat_b, gn_gamma, gn_beta, gn_alpha,
           lin_W, lin_b):
    params = dict(W0=np.asarray(W0), b0=np.asarray(b0),
                  gn0_gamma=np.asarray(gn0_gamma), gn0_beta=np.asarray(gn0_beta),
                  gn0_alpha=np.asarray(gn0_alpha), gat_W=np.asarray(gat_W),
                  gat_att_src=np.asarray(gat_att_src),
                  gat_att_dst=np.asarray(gat_att_dst), gat_b=np.asarray(gat_b),
                  gn_gamma=np.asarray(gn_gamma), gn_beta=np.asarray(gn_beta),
                  gn_alpha=np.asarray(gn_alpha), lin_W=np.asarray(lin_W),
                  lin_b=np.asarray(lin_b))
    x1 = np.asarray(x1, np.float32)
    x2 = np.asarray(x2, np.float32)
    edge_index1 = np.asarray(edge_index1)
    edge_index2 = np.asarray(edge_index2)
    batch1 = np.asarray(batch1)
    batch2 = np.asarray(batch2)
    runner, in_maps, nloc, nloc_pad, f_out = _get_runner_and_inmaps(
        x1, x2, edge_index1, edge_index2, batch1, batch2, params)
    runner.stage(in_maps)
    res = runner.results(runner.run())
    n = x1.shape[0]
    out = np.zeros((2, n, f_out), np.float32)
    for c in range(NCORES):
        out[0, c * nloc:(c + 1) * nloc] = res[c]["out1"][:nloc]
        out[1, c * nloc:(c + 1) * nloc] = res[c]["out2"][:nloc]
    return out
